# revision 6
# baseline (speedup 1.0000x reference)
"""Causal multi-head attention on 8 Trainium2 NeuronCores.

Problem: x[4,2048,1024] @ w_qkv[1024,3072] -> causal MHA (16 heads, hd=64) -> @ w_out.

Sharding: batch (4) x head-group (2 x 8 heads) = 8 cores. Each core:
  phase 1: QKV projection for its batch + its 8 heads (bf16, K=128 chunks).
  phase 2: causal attention per HEAD PAIR: the S^T matmuls of heads (2p,
           2p+1) live on complementary 64-partition halves of the qkt tiles,
           so they lower to (64,128) row-tiled PE instructions at
           tile_position (0,0)/(64,0) and execute CONCURRENTLY on disjoint
           PE subarray row-groups (2x S throughput vs the serial per-head
           chain). Outputs go to different PSUM banks of one [128,1024]
           tile; one strided exp covers both heads. A matmuls stay K=128
           ([V|1] with ones column for the softmax denominator).
  phase 3: partial output projection; host sums the two partials per batch.

PSUM (8 banks): tag "S" ss pairs [128,1024]x2 = 4; tag "P" proj/outproj
[128,512]x2 = 2; tag "A" pa [65,512]x2 = 2. Prologue runs the first
quarter's projection chunk-major in rounds of 2 groups so matmuls start as
soon as chunk 0 lands from HBM and the PE stream stays dense (HAM warm).

Softmax: exp on ACT (scale 0.125, no max subtraction: |s| <~ 6), masked
(future) triangle zeroed post-exp by a multiply on diagonal blocks,
normalization via reciprocal + gpsimd partition_broadcast + DVE multiply.
Output is written bf16 (host reduction upcasts to f32).
"""
import sys

if "/opt/trn_rl_repo" not in sys.path:
    sys.path.insert(0, "/opt/trn_rl_repo")

import ml_dtypes
import numpy as np

import concourse.tile as tile
from concourse import bacc, mybir
from concourse.bass_utils import run_bass_kernel_spmd

F32 = mybir.dt.float32
BF16 = mybir.dt.bfloat16
EXP = mybir.ActivationFunctionType.Exp

B, T, C, H = 4, 2048, 1024, 16
HD = C // H              # 64
HPC = 8                  # heads per core
CPC = HPC * HD           # 512 channels per core
NCHUNK = C // 128        # 8 contraction chunks of 128
NQ = 4                   # t-quarters (512 each) for phase-1 x streaming
TQ = T // NQ             # 512
NKB = T // 128           # 16 key blocks
NCT = CPC // 128         # 4 c'-tiles per projection (q and k each)
NPAIR = HPC // 2         # 4 head pairs per core

_NC_CACHE = None


def _build_nc():
    """Build the SPMD program (identical on all 8 cores)."""
    nc = bacc.Bacc()

    wqkv = nc.dram_tensor("wqkv", [NCHUNK, 128, 3 * CPC], BF16, kind="ExternalInput")
    xq = nc.dram_tensor("xq", [NQ, NCHUNK, 128, TQ], BF16, kind="ExternalInput")
    wo = nc.dram_tensor("wo", [NCT, 128, C], BF16, kind="ExternalInput")
    maskneg = nc.dram_tensor("maskneg", [128, 128], BF16, kind="ExternalInput")
    out = nc.dram_tensor("out", [T, C], BF16, kind="ExternalOutput")

    with tile.TileContext(nc) as tc, \
         tc.tile_pool(name="pers", bufs=1) as pers, \
         tc.tile_pool(name="xpool", bufs=2) as xpool, \
         tc.tile_pool(name="epool", bufs=8) as epool, \
         tc.tile_pool(name="npool", bufs=4) as npool, \
         tc.tile_pool(name="opool", bufs=4) as opool, \
         tc.tile_pool(name="psum", bufs=1, space="PSUM") as psum:
        # persistent SBUF
        qkt = [pers.tile([128, T], BF16, name=f"qkt{i}") for i in range(2 * NCT)]
        vsb = pers.tile([128, NKB * (CPC + HPC)], BF16, name="vsb")  # 16 x (8 x 65)
        atn = [pers.tile([128, T], BF16, name=f"atn{i}") for i in range(NCT)]
        mask_sb = pers.tile([128, 128], BF16, name="mask_sb")
        nc.sync.dma_start(mask_sb[:], maskneg[:, :])
        # ones columns of [V|1]: memset f32 staging, strided DVE copy (casts to bf16)
        ones_sb = pers.tile([128, NKB * HPC], F32, name="ones_sb")
        nc.vector.memset(ones_sb[:], 1.0)
        nc.vector.tensor_copy(
            vsb.rearrange("p (t h e) -> p (t h) e", h=HPC, e=HD + 1)[:, :, HD:HD + 1],
            ones_sb[:, :, None],
        )
        w_sb = [None] * NCHUNK
        wo_sb = []
        xt_cur = [None] * NCHUNK

        def load_x(tq, c=None):
            for cc in (range(NCHUNK) if c is None else [c]):
                x_t = xpool.tile([128, TQ], BF16, name=f"x{cc}", tag=f"x{cc}")
                nc.sync.dma_start(x_t[:], xq[tq, cc])
                xt_cur[cc] = x_t

        def load_w(c):
            wt = pers.tile([128, 3 * CPC], BF16, name=f"w{c}", uniquify=False)
            nc.sync.dma_start(wt[:], wqkv[c])
            w_sb[c] = wt

        def proj_mm(tq, g, c, ps, xt):
            """Chunk c's matmul for projection group g (g: 8 QK + 4 V)."""
            st = (c == 0)
            sp = (c == NCHUNK - 1)
            if g < 2 * NCT:
                nc.tensor.matmul(
                    ps[:, 0:TQ], w_sb[c][:, 128 * g:128 * (g + 1)], xt[c][:],
                    start=st, stop=sp, skip_group_check=True,
                )
            else:
                vt = g - 2 * NCT
                nc.tensor.matmul(
                    ps[:, 0:CPC], xt[c][:, 128 * vt:128 * (vt + 1)],
                    w_sb[c][:, 2 * CPC:3 * CPC],
                    start=st, stop=sp, skip_group_check=True,
                )

        def proj_evac(tq, g, ps):
            """Evacuate one projection group psum -> sbuf."""
            if g < 2 * NCT:
                nc.vector.tensor_copy(qkt[g][:, TQ * tq:TQ * (tq + 1)], ps[:, 0:TQ])
            else:
                vt = g - 2 * NCT
                ti = tq * (TQ // 128) + vt
                dst = vsb[:, (CPC + HPC) * ti:(CPC + HPC) * (ti + 1)]
                nc.vector.tensor_copy(
                    dst.rearrange("p (h e) -> p h e", e=HD + 1)[:, :, 0:HD],
                    ps[:, 0:CPC].rearrange("p (h e) -> p h e", e=HD),
                )

        def proj_unit(tq, g, xt=None):
            """One full projection group (for filler use)."""
            if xt is None:
                xt = list(xt_cur)
            ps = psum.tile([128, 512], F32, name="psp", tag="P", bufs=2)
            for c in range(NCHUNK):
                proj_mm(tq, g, c, ps, xt)
            proj_evac(tq, g, ps)

        def outproj_unit(tq, g):
            """One output tile [t128, 512]: g in 0..7 (4 t-tiles x 2 col halves)."""
            tt = tq * 4 + g // 2
            jj = g % 2
            ps = psum.tile([128, 512], F32, name="pso", tag="P", bufs=2)
            for cc in range(NCT):
                nc.tensor.matmul(
                    ps[:], atn[cc][:, 128 * tt:128 * (tt + 1)],
                    wo_sb[cc][:, 512 * jj:512 * (jj + 1)],
                    start=(cc == 0), stop=(cc == NCT - 1),
                    skip_group_check=True,
                )
            oc = opool.tile([128, 512], BF16, name="oc", tag="oc")
            nc.vector.tensor_copy(oc[:], ps[:])
            nc.sync.dma_start(
                out[128 * tt:128 * (tt + 1), 512 * jj:512 * (jj + 1)], oc[:]
            )

        def attn_pair_chain(p, j, fillers=None):
            """Causal attention for head pair (2p, 2p+1), 512-query tile j.

            Per key block kb: S^T for h0 streams on partitions 0:64 (PE tile
            T0) into ss[:, 0:n] (bank 0), h1 on 64:128 (T8) into
            ss[:, 512:512+n] (bank 1) -- concurrent row-tiled matmuls,
            disjoint banks. One strided exp covers both heads. A matmuls are
            K=128 [V|1] per head accumulating attn^T + denominator in
            [65,512] psum; normalization reads psum directly (reciprocal +
            gpsimd partition_broadcast + DVE multiply), as in the baseline.
            """
            h0, h1 = 2 * p, 2 * p + 1
            QK = ((qkt[p][0:64, :], qkt[NCT + p][0:64, :]),
                  (qkt[p][64:128, :], qkt[NCT + p][64:128, :]))
            q0 = 512 * j
            nkb = 4 * j + 4
            pa = [
                psum.tile([65, 512], F32, name=f"pa{s}", tag="A", bufs=2)
                for s in (0, 1)
            ]

            ees = [None] * nkb
            segs = []
            for kb in range(nkb):
                col0 = 0 if kb < 4 * j else 128 * (kb - 4 * j)
                segs.append((kb, col0, 512 - col0))

            def emit_s(kb):
                _, col0, n = segs[kb]
                ss = psum.tile([128, 1024], F32, name="ss", tag="S", bufs=2)
                ee = epool.tile([128, 1024], BF16, name="ee", tag="E")
                for side in (0, 1):
                    Qh, Kh = QK[side]
                    nc.tensor.matmul(
                        ss[:, 512 * side:512 * side + n],
                        Kh[:, 128 * kb:128 * (kb + 1)],
                        Qh[:, q0 + col0:q0 + 512],
                        start=True, stop=True, skip_group_check=True,
                    )
                sv = ss.rearrange("p (b n) -> p b n", b=2)[:, :, 0:n]
                ev = ee.rearrange("p (b n) -> p b n", b=2)[:, :, 0:n]
                nc.scalar.activation(ev, sv, EXP, scale=0.125)
                if kb >= 4 * j:  # zero the masked (future) triangle post-exp
                    for side in (0, 1):
                        nc.vector.tensor_mul(
                            ee[:, 512 * side:512 * side + 128],
                            ee[:, 512 * side:512 * side + 128],
                            mask_sb[:],
                        )
                ees[kb] = ee

            def emit_a(kb):
                _, col0, n = segs[kb]
                ee = ees[kb]
                st = (kb == 0)
                sp = (kb == nkb - 1)
                for side, h in ((0, h0), (1, h1)):
                    vbase = (CPC + HPC) * kb + (HD + 1) * h
                    nc.tensor.matmul(
                        pa[side][:, col0:512],
                        vsb[:, vbase:vbase + HD + 1],
                        ee[:, 512 * side:512 * side + n],
                        start=st, stop=sp, skip_group_check=True,
                    )

            LAG = 2
            for kb in range(nkb + LAG):
                if kb >= LAG:
                    emit_a(kb - LAG)
                if fillers and kb % 2 == 1:
                    fillers.pop(0)()
                if kb < nkb:
                    emit_s(kb)
            # normalize each head: den row 64 of pa -> reciprocal -> bcast -> mul
            for side, h in ((0, h0), (1, h1)):
                den = npool.tile([1, 512], F32, name="den", tag="den")
                nc.vector.tensor_copy(den[:], pa[side][HD:HD + 1, :])
                rec = npool.tile([1, 512], F32, name="rec", tag="rec")
                nc.vector.reciprocal_approx_fast(rec[:], den[:])
                bc = npool.tile([HD, 512], F32, name="bc", tag="bc")
                nc.gpsimd.partition_broadcast(bc[:], rec[:])
                r0 = HD * (h % 2)
                nc.vector.tensor_mul(
                    atn[p][r0:r0 + HD, q0:q0 + 512], pa[side][0:HD, :], bc[:]
                )

        # ---- pipelined schedule ----
        # prologue: interleave w/x chunk DMAs; chunk-major proj rounds of 2
        # groups so matmuls start as soon as chunk 0 lands and stay dense.
        for c in range(NCHUNK):
            load_w(c)
            load_x(0, c)
        for cc in range(NCT):
            wt = pers.tile([128, C], BF16, name=f"wo{cc}")
            nc.sync.dma_start(wt[:], wo[cc])
            wo_sb.append(wt)
        xts0 = list(xt_cur)
        for r in range(6):  # rounds of 2 groups, chunk-major
            pss = [
                psum.tile([128, 512], F32, name="psp", tag="P", bufs=2)
                for _ in range(2)
            ]
            for c in range(NCHUNK):
                for i, g in enumerate(range(2 * r, 2 * r + 2)):
                    proj_mm(0, g, c, pss[i], xts0)
            for i, g in enumerate(range(2 * r, 2 * r + 2)):
                proj_evac(0, g, pss[i])
        load_x(1)
        for tq in range(1, NQ + 1):
            j = tq - 1
            fillers = []
            if tq < NQ:
                xts = list(xt_cur)
                for g in range(12):
                    fillers.append((lambda tq=tq, g=g, xts=xts: proj_unit(tq, g, xts)))
            if j >= 1:
                for g in range(8):
                    fillers.append((lambda j=j, g=g: outproj_unit(j - 1, g)))
            for p in range(NPAIR):
                attn_pair_chain(p, j, fillers)
                if p == 1 and tq + 1 < NQ:
                    load_x(tq + 1)
            while fillers:
                fillers.pop(0)()
            if tq == NQ:
                for g in range(8):
                    outproj_unit(j, g)
    nc.finalize()
    return nc


def _prep_inputs(x, w_qkv, w_out):
    """Shard + pack host-side: returns in_maps for cores 0..7 (core = 2*b + g)."""
    in_maps = []
    maskneg = np.where(
        np.arange(128)[None, :] >= np.arange(128)[:, None], 1.0, 0.0
    ).astype(ml_dtypes.bfloat16)
    for b in range(B):
        xT = np.ascontiguousarray(x[b].T)  # [C, T]
        xq_bf = np.ascontiguousarray(
            xT.reshape(NCHUNK, 128, NQ, TQ).transpose(2, 0, 1, 3)
        ).astype(ml_dtypes.bfloat16)  # [NQ, NCHUNK, 128, TQ]
        for g in range(2):
            wq = w_qkv[:, CPC * g:CPC * (g + 1)]
            wk = w_qkv[:, C + CPC * g:C + CPC * (g + 1)]
            wv = w_qkv[:, 2 * C + CPC * g:2 * C + CPC * (g + 1)]
            wqkv_pack = np.concatenate([wq, wk, wv], axis=1).reshape(
                NCHUNK, 128, 3 * CPC
            )
            wo_pack = np.ascontiguousarray(
                w_out[CPC * g:CPC * (g + 1), :].reshape(NCT, 128, C)
            )
            in_maps.append({
                "wqkv": np.ascontiguousarray(wqkv_pack).astype(ml_dtypes.bfloat16),
                "xq": xq_bf,
                "wo": wo_pack.astype(ml_dtypes.bfloat16),
                "maskneg": maskneg,
            })
    return in_maps


def run(x, w_qkv, w_out, trace=False, trace_cores=None):
    global _NC_CACHE
    if _NC_CACHE is None:
        _NC_CACHE = _build_nc()
    in_maps = _prep_inputs(x, w_qkv, w_out)
    res = run_bass_kernel_spmd(
        _NC_CACHE, in_maps, list(range(8)),
        trace=trace, trace_cores=trace_cores,
    )
    outs = [np.asarray(res.results[i]["out"], np.float32) for i in range(8)]
    full = np.empty((B, T, C), np.float32)
    for b in range(B):
        full[b] = outs[2 * b] + outs[2 * b + 1]
    return full, res


def kernel(x, w_qkv, w_out):
    x = np.asarray(x, np.float32)
    w_qkv = np.asarray(w_qkv, np.float32)
    w_out = np.asarray(w_out, np.float32)
    full, _ = run(x, w_qkv, w_out)
    return full


# revision 8
# speedup vs baseline: 1.0327x; 1.0327x over previous
"""Causal multi-head attention on 8 Trainium2 NeuronCores.

Problem: x[4,2048,1024] @ w_qkv[1024,3072] -> causal MHA (16 heads, hd=64) -> @ w_out.

Sharding: batch (4) x head-group (2 x 8 heads) = 8 cores. Each core:
  phase 1: QKV projection for its batch + its 8 heads (bf16, K=128 chunks).
  phase 2: causal attention per HEAD PAIR: the S^T matmuls of heads (2p,
           2p+1) live on complementary 64-partition halves of the qkt tiles,
           so they lower to (64,128) row-tiled PE instructions at
           tile_position (0,0)/(64,0) and execute CONCURRENTLY on disjoint
           PE subarray row-groups (2x S throughput vs the serial per-head
           chain). Outputs go to different PSUM banks of one [128,1024]
           tile; one strided exp covers both heads. A matmuls stay K=128
           ([V|1] with ones column for the softmax denominator).
  phase 3: partial output projection; host sums the two partials per batch.

PSUM (8 banks): tag "S" ss pairs [128,1024]x2 = 4; tag "P" proj/outproj
[128,512]x2 = 2; tag "A" pa [65,512]x2 = 2. Prologue runs the first
quarter's projection chunk-major in rounds of 2 groups so matmuls start as
soon as chunk 0 lands from HBM and the PE stream stays dense (HAM warm).

Softmax: exp on ACT (scale 0.125, no max subtraction: |s| <~ 6), masked
(future) triangle zeroed post-exp by a multiply on diagonal blocks,
normalization via reciprocal + gpsimd partition_broadcast + DVE multiply.
Output is written bf16 (host reduction upcasts to f32).
"""
import sys

if "/opt/trn_rl_repo" not in sys.path:
    sys.path.insert(0, "/opt/trn_rl_repo")

import ml_dtypes
import numpy as np

import concourse.tile as tile
from concourse import bacc, mybir
from concourse.bass_utils import run_bass_kernel_spmd

F32 = mybir.dt.float32
BF16 = mybir.dt.bfloat16
EXP = mybir.ActivationFunctionType.Exp

B, T, C, H = 4, 2048, 1024, 16
HD = C // H              # 64
HPC = 8                  # heads per core
CPC = HPC * HD           # 512 channels per core
NCHUNK = C // 128        # 8 contraction chunks of 128
NQ = 4                   # t-quarters (512 each) for phase-1 x streaming
TQ = T // NQ             # 512
NKB = T // 128           # 16 key blocks
NCT = CPC // 128         # 4 c'-tiles per projection (q and k each)
NPAIR = HPC // 2         # 4 head pairs per core

_NC_CACHE = None


def _build_nc():
    """Build the SPMD program (identical on all 8 cores)."""
    nc = bacc.Bacc()

    wqkv = nc.dram_tensor("wqkv", [NCHUNK, 128, 3 * CPC], BF16, kind="ExternalInput")
    xq = nc.dram_tensor("xq", [NQ, NCHUNK, 128, TQ], BF16, kind="ExternalInput")
    wo = nc.dram_tensor("wo", [NCT, 128, C], BF16, kind="ExternalInput")
    maskneg = nc.dram_tensor("maskneg", [128, 128], BF16, kind="ExternalInput")
    out = nc.dram_tensor("out", [T, C], BF16, kind="ExternalOutput")

    with tile.TileContext(nc) as tc, \
         tc.tile_pool(name="pers", bufs=1) as pers, \
         tc.tile_pool(name="xpool", bufs=2) as xpool, \
         tc.tile_pool(name="epool", bufs=8) as epool, \
         tc.tile_pool(name="npool", bufs=4) as npool, \
         tc.tile_pool(name="opool", bufs=4) as opool, \
         tc.tile_pool(name="psum", bufs=1, space="PSUM") as psum:
        # persistent SBUF
        qkt = [pers.tile([128, T], BF16, name=f"qkt{i}") for i in range(2 * NCT)]
        vsb = pers.tile([128, NKB * (CPC + HPC)], BF16, name="vsb")  # 16 x (8 x 65)
        atn = [pers.tile([128, T], BF16, name=f"atn{i}") for i in range(NCT)]
        mask_sb = pers.tile([128, 128], BF16, name="mask_sb")
        nc.sync.dma_start(mask_sb[:], maskneg[:, :])
        # ones columns of [V|1]: memset f32 staging, strided DVE copy (casts to bf16)
        ones_sb = pers.tile([128, NKB * HPC], F32, name="ones_sb")
        nc.vector.memset(ones_sb[:], 1.0)
        nc.vector.tensor_copy(
            vsb.rearrange("p (t h e) -> p (t h) e", h=HPC, e=HD + 1)[:, :, HD:HD + 1],
            ones_sb[:, :, None],
        )
        w_sb = [None] * NCHUNK
        wo_sb = []
        xt_cur = [None] * NCHUNK

        def load_x(tq, c=None):
            for cc in (range(NCHUNK) if c is None else [c]):
                x_t = xpool.tile([128, TQ], BF16, name=f"x{cc}", tag=f"x{cc}")
                nc.sync.dma_start(x_t[:], xq[tq, cc])
                xt_cur[cc] = x_t

        def load_w(c):
            wt = pers.tile([128, 3 * CPC], BF16, name=f"w{c}", uniquify=False)
            nc.sync.dma_start(wt[:], wqkv[c])
            w_sb[c] = wt

        def proj_mm(tq, g, c, ps, xt):
            """Chunk c's matmul for projection group g (g: 8 QK + 4 V)."""
            st = (c == 0)
            sp = (c == NCHUNK - 1)
            if g < 2 * NCT:
                nc.tensor.matmul(
                    ps[:, 0:TQ], w_sb[c][:, 128 * g:128 * (g + 1)], xt[c][:],
                    start=st, stop=sp, skip_group_check=True,
                )
            else:
                vt = g - 2 * NCT
                nc.tensor.matmul(
                    ps[:, 0:CPC], xt[c][:, 128 * vt:128 * (vt + 1)],
                    w_sb[c][:, 2 * CPC:3 * CPC],
                    start=st, stop=sp, skip_group_check=True,
                )

        def proj_evac(tq, g, ps):
            """Evacuate one projection group psum -> sbuf."""
            if g < 2 * NCT:
                nc.vector.tensor_copy(qkt[g][:, TQ * tq:TQ * (tq + 1)], ps[:, 0:TQ])
            else:
                vt = g - 2 * NCT
                ti = tq * (TQ // 128) + vt
                dst = vsb[:, (CPC + HPC) * ti:(CPC + HPC) * (ti + 1)]
                nc.vector.tensor_copy(
                    dst.rearrange("p (h e) -> p h e", e=HD + 1)[:, :, 0:HD],
                    ps[:, 0:CPC].rearrange("p (h e) -> p h e", e=HD),
                )

        def proj_unit(tq, g, xt=None):
            """One full projection group (for filler use)."""
            if xt is None:
                xt = list(xt_cur)
            ps = psum.tile([128, 512], F32, name="psp", tag="P", bufs=2)
            for c in range(NCHUNK):
                proj_mm(tq, g, c, ps, xt)
            proj_evac(tq, g, ps)

        def outproj_unit(tq, g):
            """One output tile [t128, 512]: g in 0..7 (4 t-tiles x 2 col halves)."""
            tt = tq * 4 + g // 2
            jj = g % 2
            ps = psum.tile([128, 512], F32, name="pso", tag="P", bufs=2)
            for cc in range(NCT):
                nc.tensor.matmul(
                    ps[:], atn[cc][:, 128 * tt:128 * (tt + 1)],
                    wo_sb[cc][:, 512 * jj:512 * (jj + 1)],
                    start=(cc == 0), stop=(cc == NCT - 1),
                    skip_group_check=True,
                )
            oc = opool.tile([128, 512], BF16, name="oc", tag="oc")
            nc.vector.tensor_copy(oc[:], ps[:])
            nc.sync.dma_start(
                out[128 * tt:128 * (tt + 1), 512 * jj:512 * (jj + 1)], oc[:]
            )

        def attn_pair_chain(p, j, fillers=None, debt=None):
            """Causal attention for head pair (2p, 2p+1), 512-query tile j.

            Per key block kb: S^T for h0 streams on partitions 0:64 (PE tile
            T0) into ss[:, 0:n] (bank 0), h1 on 64:128 (T8) into
            ss[:, 512:512+n] (bank 1) -- concurrent row-tiled matmuls,
            disjoint banks. One strided exp covers both heads. A matmuls are
            K=128 [V|1] per head accumulating attn^T + denominator in
            [65,512] psum; normalization reads psum directly (reciprocal +
            gpsimd partition_broadcast + DVE multiply), as in the baseline.
            """
            h0, h1 = 2 * p, 2 * p + 1
            QK = ((qkt[p][0:64, :], qkt[NCT + p][0:64, :]),
                  (qkt[p][64:128, :], qkt[NCT + p][64:128, :]))
            q0 = 512 * j
            nkb = 4 * j + 4
            pa = [
                psum.tile([65, 512], F32, name=f"pa{s}", tag="A", bufs=2)
                for s in (0, 1)
            ]

            ees = [None] * nkb
            segs = []
            for kb in range(nkb):
                col0 = 0 if kb < 4 * j else 128 * (kb - 4 * j)
                segs.append((kb, col0, 512 - col0))

            def emit_s(kb):
                _, col0, n = segs[kb]
                ss = psum.tile([128, 1024], F32, name="ss", tag="S", bufs=2)
                ee = epool.tile([128, 1024], BF16, name="ee", tag="E")
                for side in (0, 1):
                    Qh, Kh = QK[side]
                    nc.tensor.matmul(
                        ss[:, 512 * side:512 * side + n],
                        Kh[:, 128 * kb:128 * (kb + 1)],
                        Qh[:, q0 + col0:q0 + 512],
                        start=True, stop=True, skip_group_check=True,
                    )
                sv = ss.rearrange("p (b n) -> p b n", b=2)[:, :, 0:n]
                ev = ee.rearrange("p (b n) -> p b n", b=2)[:, :, 0:n]
                nc.scalar.activation(ev, sv, EXP, scale=0.125)
                if kb >= 4 * j:  # zero the masked (future) triangle post-exp
                    for side in (0, 1):
                        nc.vector.tensor_mul(
                            ee[:, 512 * side:512 * side + 128],
                            ee[:, 512 * side:512 * side + 128],
                            mask_sb[:],
                        )
                ees[kb] = ee

            def emit_a(kb):
                _, col0, n = segs[kb]
                ee = ees[kb]
                st = (kb == 0)
                sp = (kb == nkb - 1)
                for side, h in ((0, h0), (1, h1)):
                    vbase = (CPC + HPC) * kb + (HD + 1) * h
                    nc.tensor.matmul(
                        pa[side][:, col0:512],
                        vsb[:, vbase:vbase + HD + 1],
                        ee[:, 512 * side:512 * side + n],
                        start=st, stop=sp, skip_group_check=True,
                    )

            LAG = 2
            for kb in range(nkb + LAG):
                if kb >= LAG:
                    emit_a(kb - LAG)
                if kb < nkb:
                    n = segs[kb][2]
                    debt[0] += 0.44 * n + 140  # ACT-PE gap per unit (ns)
                while fillers and debt[0] >= fillers[0][0]:
                    cost, fn = fillers.pop(0)
                    debt[0] -= cost
                    fn()
                if kb < nkb:
                    emit_s(kb)
            # normalize each head: den row 64 of pa -> reciprocal -> bcast -> mul
            for side, h in ((0, h0), (1, h1)):
                den = npool.tile([1, 512], F32, name="den", tag="den")
                nc.vector.tensor_copy(den[:], pa[side][HD:HD + 1, :])
                rec = npool.tile([1, 512], F32, name="rec", tag="rec")
                nc.vector.reciprocal_approx_fast(rec[:], den[:])
                bc = npool.tile([HD, 512], F32, name="bc", tag="bc")
                nc.gpsimd.partition_broadcast(bc[:], rec[:])
                r0 = HD * (h % 2)
                nc.vector.tensor_mul(
                    atn[p][r0:r0 + HD, q0:q0 + 512], pa[side][0:HD, :], bc[:]
                )

        # ---- pipelined schedule ----
        # prologue: interleave w/x chunk DMAs; chunk-major proj rounds of 2
        # groups so matmuls start as soon as chunk 0 lands and stay dense.
        for c in range(NCHUNK):
            load_w(c)
            load_x(0, c)
        for cc in range(NCT):
            wt = pers.tile([128, C], BF16, name=f"wo{cc}")
            nc.sync.dma_start(wt[:], wo[cc])
            wo_sb.append(wt)
        xts0 = list(xt_cur)
        for r in range(6):  # rounds of 2 groups, chunk-major
            pss = [
                psum.tile([128, 512], F32, name="psp", tag="P", bufs=2)
                for _ in range(2)
            ]
            for c in range(NCHUNK):
                for i, g in enumerate(range(2 * r, 2 * r + 2)):
                    proj_mm(0, g, c, pss[i], xts0)
            for i, g in enumerate(range(2 * r, 2 * r + 2)):
                proj_evac(0, g, pss[i])
        load_x(1)
        PC, OC = 1730, 914  # filler PE cost (ns): proj group, outproj unit

        def pf(tq, g, xts):
            return (PC, lambda: proj_unit(tq, g, xts))

        def of(tq, g):
            return (OC, lambda: outproj_unit(tq, g))

        xts_q = {}
        for tq in range(1, NQ + 1):
            j = tq - 1
            if tq < NQ:
                xts_q[tq] = list(xt_cur)
            # filler plans: proj(3) groups for pair p of j=3 must complete
            # before chain(p, 3); its V groups before emit_a(12) of chain 0.
            if j == 0:
                fillers = [pf(1, g, xts_q[1]) for g in range(12)]
            elif j == 1:
                fillers = [pf(2, g, xts_q[2]) for g in range(12)]
                fillers += [of(0, g) for g in range(8)]
            elif j == 2:
                fillers = [pf(3, g, xts_q[3]) for g in (0, 4, 8, 9, 10, 11)]
                fillers += [of(1, g) for g in range(8)]
            else:
                fillers = [pf(3, g, xts_q[3]) for g in (1, 5)]
                fillers += [of(2, 0), of(2, 1)]
                fillers += [pf(3, g, xts_q[3]) for g in (2, 6)]
                fillers += [of(2, 2), of(2, 3)]
                fillers += [pf(3, g, xts_q[3]) for g in (3, 7)]
                fillers += [of(2, g) for g in (4, 5, 6, 7)]
            debt = [0.0]
            for p in range(NPAIR):
                attn_pair_chain(p, j, fillers, debt)
                if p == 1 and tq + 1 < NQ:
                    load_x(tq + 1)
            while fillers:  # leftovers run dense before the next quarter
                fillers.pop(0)[1]()
            if tq == NQ:
                for g in range(8):
                    outproj_unit(j, g)
    nc.finalize()
    return nc


def _prep_inputs(x, w_qkv, w_out):
    """Shard + pack host-side: returns in_maps for cores 0..7 (core = 2*b + g)."""
    in_maps = []
    maskneg = np.where(
        np.arange(128)[None, :] >= np.arange(128)[:, None], 1.0, 0.0
    ).astype(ml_dtypes.bfloat16)
    for b in range(B):
        xT = np.ascontiguousarray(x[b].T)  # [C, T]
        xq_bf = np.ascontiguousarray(
            xT.reshape(NCHUNK, 128, NQ, TQ).transpose(2, 0, 1, 3)
        ).astype(ml_dtypes.bfloat16)  # [NQ, NCHUNK, 128, TQ]
        for g in range(2):
            wq = w_qkv[:, CPC * g:CPC * (g + 1)]
            wk = w_qkv[:, C + CPC * g:C + CPC * (g + 1)]
            wv = w_qkv[:, 2 * C + CPC * g:2 * C + CPC * (g + 1)]
            wqkv_pack = np.concatenate([wq, wk, wv], axis=1).reshape(
                NCHUNK, 128, 3 * CPC
            )
            wo_pack = np.ascontiguousarray(
                w_out[CPC * g:CPC * (g + 1), :].reshape(NCT, 128, C)
            )
            in_maps.append({
                "wqkv": np.ascontiguousarray(wqkv_pack).astype(ml_dtypes.bfloat16),
                "xq": xq_bf,
                "wo": wo_pack.astype(ml_dtypes.bfloat16),
                "maskneg": maskneg,
            })
    return in_maps


def run(x, w_qkv, w_out, trace=False, trace_cores=None):
    global _NC_CACHE
    if _NC_CACHE is None:
        _NC_CACHE = _build_nc()
    in_maps = _prep_inputs(x, w_qkv, w_out)
    res = run_bass_kernel_spmd(
        _NC_CACHE, in_maps, list(range(8)),
        trace=trace, trace_cores=trace_cores,
    )
    outs = [np.asarray(res.results[i]["out"], np.float32) for i in range(8)]
    full = np.empty((B, T, C), np.float32)
    for b in range(B):
        full[b] = outs[2 * b] + outs[2 * b + 1]
    return full, res


def kernel(x, w_qkv, w_out):
    x = np.asarray(x, np.float32)
    w_qkv = np.asarray(w_qkv, np.float32)
    w_out = np.asarray(w_out, np.float32)
    full, _ = run(x, w_qkv, w_out)
    return full
